# revision 32
# baseline (speedup 1.0000x reference)
"""Trainium2 Bass kernel for a multi-head attention block (B=4, S=2048, D=1024, H=16).

Sharding over 8 NeuronCores: core c handles batch b=c//2 and head-group
hg=c%2 (8 of 16 heads). Each core computes its heads' QKV projections,
causal attention, and a partial output projection (row-sharded Wo); the
2-way reduction per batch (the "all-reduce after w_o") happens on host at
gather time, along with the bo bias (bo/2 added on each device).

Device dataflow (all bf16 matmuls, fp32 PSUM accumulate; activations kept
transposed, feature-on-partition). The emission is software-pipelined:
QKV projections for later blocks and the Wo projection for the previous
block are chopped into small "filler" units (4 matmuls each) that are
interleaved between the attention chunks of the current block, so the
tensor engine never stalls on the scalar engine's softmax exp (exp of a
full chunk costs ~1.1us vs ~0.65us of tensor work per chunk). Because
the causal attention work grows with the block index while filler supply
shrinks, the k/v projections of the last block are deferred into the
last window (legal: its attention only needs them for the diagonal
chunks) to keep the tensor engine fed there. Attention per chunk:
scoresT[k,q] for a head pair land in one 2-bank PSUM tile via two
concurrent K=64 row-group matmuls, one merged exp on ScalarE produces
bf16 probs, diagonal blocks are causal-trimmed by qoff and masked by a
binary triangle multiply post-exp, PV matmuls accumulate [65, 512] per
head (row 64 = softmax denominator via a ones column in v), then
reciprocal + partition-broadcast + multiply normalize into paired
[128, 512] tiles feeding the K=128 Wo projection fillers of the next
window. Weights and activations load via single 3-dim-AP DMAs (one per
4-chunk group) to avoid per-chunk DMA issue serialization at startup;
K/Q bias moves run on ScalarE (identity+bias, same act table as exp)
to keep VectorE off the PSUM-free critical path.
"""

import numpy as np
import ml_dtypes
from contextlib import ExitStack

import concourse.bass as bass
import concourse.tile as tile
from concourse import bacc, mybir
from concourse.bass_utils import run_bass_kernel_spmd
from concourse.alu_op_type import AluOpType

F32 = mybir.dt.float32
BF16 = mybir.dt.bfloat16

S = 2048          # sequence length
D = 1024          # model dim
NH = 8            # heads per core
DKH = 64          # head dim
NHP = 4           # head pairs per core
SB = 512          # seq block (q block)
NSB = S // SB     # 4
KC = 128          # k chunk
NDIN = D // 128   # 8 input-dim chunks
VW = DKH + 1      # 65: v columns per head incl. ones column


def build_program():
    nc = bacc.Bacc("TRN2", target_bir_lowering=False, debug=False, num_devices=8)

    xq = nc.dram_tensor("xq_t", [D, S], BF16, kind="ExternalInput").ap()
    xk = nc.dram_tensor("xk_t", [D, S], BF16, kind="ExternalInput").ap()
    xv = nc.dram_tensor("xv_t", [D, S], BF16, kind="ExternalInput").ap()
    wq = nc.dram_tensor("wq_t", [D, 512], BF16, kind="ExternalInput").ap()
    wkv = nc.dram_tensor("wkv_t", [D, 1024], BF16, kind="ExternalInput").ap()
    wo = nc.dram_tensor("wo_t", [512, D], BF16, kind="ExternalInput").ap()
    bqs = nc.dram_tensor("bq_s", [128, 4], F32, kind="ExternalInput").ap()
    bks = nc.dram_tensor("bk_s", [128, 4], F32, kind="ExternalInput").ap()
    bvb_d = nc.dram_tensor("bv_b", [128, 512], BF16, kind="ExternalInput").ap()
    bob_d = nc.dram_tensor("bo_b", [128, D], BF16, kind="ExternalInput").ap()
    tri_d = nc.dram_tensor("tri_b", [128, 256], BF16, kind="ExternalInput").ap()
    y = nc.dram_tensor("y", [S, D], BF16, kind="ExternalOutput").ap()

    with tile.TileContext(nc) as tc, ExitStack() as ctx:
        const = ctx.enter_context(tc.tile_pool(name="const", bufs=1))
        wpool = ctx.enter_context(tc.tile_pool(name="w", bufs=1))
        acts = ctx.enter_context(tc.tile_pool(name="acts", bufs=1))
        qpool = ctx.enter_context(tc.tile_pool(name="qpool", bufs=2))
        xs = ctx.enter_context(tc.tile_pool(name="xs", bufs=2))
        probs = ctx.enter_context(tc.tile_pool(name="probs", bufs=4))
        atp = ctx.enter_context(tc.tile_pool(name="at", bufs=2))
        nrm = ctx.enter_context(tc.tile_pool(name="nrm", bufs=4))
        ysb = ctx.enter_context(tc.tile_pool(name="ysb", bufs=4))
        mmps = ctx.enter_context(tc.tile_pool(name="mmps", bufs=2, space="PSUM"))
        scps = ctx.enter_context(tc.tile_pool(name="scps", bufs=2, space="PSUM"))
        pvps = ctx.enter_context(tc.tile_pool(name="pvps", bufs=2, space="PSUM"))

        # trigger the gpsimd custom-op library load immediately
        dum1 = const.tile([1, 16], F32, name="dum1", tag="dum1")
        nc.vector.memset(dum1[:], 0.0)
        dum2 = const.tile([8, 16], F32, name="dum2", tag="dum2")
        nc.gpsimd.partition_broadcast(dum2[:], dum1[:])

        # q/k biases first on the scalar queue (tiny, host-transposed [128,4])
        bqt4 = const.tile([128, 4], F32, name="bqt4", tag="bqt4")
        nc.scalar.dma_start(bqt4[:], bqs)
        bkt4 = const.tile([128, 4], F32, name="bkt4", tag="bkt4")
        nc.scalar.dma_start(bkt4[:], bks)
        bq_t = [bqt4[:, m_i:m_i + 1] for m_i in range(4)]
        bk_t = [bkt4[:, m_i:m_i + 1] for m_i in range(4)]

        # ---- weights: one 3-dim DMA per 4-chunk group (precise deps,
        # no per-chunk DMA issue serialization), all on the scalar queue;
        # wo is deferred to the first window (needed only from W1).
        def wtile(name, src3, eng, w=512):
            t = wpool.tile([128, 4 * w], BF16, name=name, tag=name)
            eng.dma_start(t[:].rearrange("p (c d) -> p c d", c=4), src3)
            return t[:].rearrange("p (c d) -> p c d", c=4)

        wkv3 = wkv.rearrange("(c p) d -> c p d", c=NDIN)
        wq3 = wq.rearrange("(c p) d -> c p d", c=NDIN)
        wo3 = wo.rearrange("(c p) d -> c p d", c=NHP)
        wkvk_a1 = wtile("wkvka1", wkv3[0:4, :, 0:256].rearrange("c p d -> p c d"),
                        nc.sync, w=256)
        wkvk_b1 = wtile("wkvkb1", wkv3[4:8, :, 0:256].rearrange("c p d -> p c d"),
                        nc.scalar, w=256)
        wkvk_a2 = wtile("wkvka2", wkv3[0:4, :, 256:512].rearrange("c p d -> p c d"),
                        nc.sync, w=256)
        wkvk_b2 = wtile("wkvkb2", wkv3[4:8, :, 256:512].rearrange("c p d -> p c d"),
                        nc.scalar, w=256)
        wq_a = wtile("wqa", wq3[0:4, :, :].rearrange("c p d -> p c d"), nc.scalar)
        wq_b = wtile("wqb", wq3[4:8, :, :].rearrange("c p d -> p c d"), nc.scalar)
        wkvv_a = wtile("wkvva", wkv3[0:4, :, 512:1024].rearrange("c p d -> p c d"),
                       nc.scalar)
        wkvv_b = wtile("wkvvb", wkv3[4:8, :, 512:1024].rearrange("c p d -> p c d"),
                       nc.scalar)
        wot_t = wpool.tile([128, 4 * 1024], BF16, name="wot", tag="wot")
        wot = wot_t[:].rearrange("p (c d) -> p c d", c=4)

        def wkvkm(c, m_i):
            if c < 4:
                t = wkvk_a1 if m_i < 2 else wkvk_a2
            else:
                t = wkvk_b1 if m_i < 2 else wkvk_b2
            mo = (m_i % 2) * 128
            return t[:, c % 4, mo:mo + 128]

        def wkvv(c):
            return (wkvv_a if c < 4 else wkvv_b)[:, c % 4, :]

        def wqc(c):
            return (wq_a if c < 4 else wq_b)[:, c % 4, :]

        def woh2(hp):
            return wot[:, hp, :]

        # ---- constants (mask + broadcast biases precomputed on host; the
        # gpsimd queue is frozen ~9-22us by the custom-op library load, so
        # nothing startup-critical may ride on it) ----
        tri = const.tile([128, 256], BF16, name="tri", tag="tri")
        bv_b = const.tile([128, 512], BF16, name="bvb", tag="bvb")
        bo_b = const.tile([128, D], BF16, name="bob", tag="bob")

        # bf16 ones row (at partition 64) for the PE-based denominator
        # broadcast at the tail
        ones_bc = const.tile([65, 64], BF16, name="onesbc", tag="onesbc")
        nc.vector.memset(ones_bc[64:65, :], 1.0)

        # ---- resident k/v activations ----
        kT = {}
        for hp in range(NHP):
            for sb in range(NSB):
                kT[(hp, sb)] = acts.tile([128, SB], BF16, name=f"kT{hp}_{sb}",
                                         tag=f"kT{hp}_{sb}")
        vt = [acts.tile([128, NH * VW], BF16, name=f"v{t_i}", tag=f"v{t_i}")
              for t_i in range(S // KC)]

        # ---- merged x loads: two 3-dim DMAs per tensor per seq block ----
        # dst[p, c, s] = dram[c*128 + p, sb*SB + s]
        def load_x(src, tag, sb):
            halves = []
            s4 = src.rearrange("(c p) s -> c p s", c=NDIN)
            for h, hname in ((0, "a"), (1, "b")):
                t = xs.tile([128, 4 * SB], BF16, name=f"{tag}{hname}{sb}",
                            tag=f"{tag}{hname}")
                t3 = t[:].rearrange("p (c s) -> p c s", c=4)
                s3 = s4[4 * h:4 * h + 4, :, sb * SB:(sb + 1) * SB]
                nc.sync.dma_start(t3, s3.rearrange("c p s -> p c s"))
                halves.append(t3)
            return halves

        # ---------- filler units ----------
        # Each unit is a closure emitting ~4 matmuls (+ a psum->sbuf move).
        qT = {}

        def make_qkv_units(sb, xkh, xvh, xqh):
            cell = {}

            def kp_a(m_i):
                ps = mmps.tile([128, SB], F32, name="mm", tag="mm")
                cell[("k", m_i)] = ps
                for c in range(4):
                    nc.tensor.matmul(ps[:], wkvkm(c, m_i),
                                     xkh[0][:, c, :], start=(c == 0), stop=False)

            def kp_b(m_i):
                ps = cell.pop(("k", m_i))
                for c in range(4):
                    nc.tensor.matmul(ps[:], wkvkm(c + 4, m_i),
                                     xkh[1][:, c, :], start=False, stop=(c == 3))
                if sb in (0, 3):
                    nc.vector.tensor_scalar_add(kT[(m_i, sb)][:], ps[:],
                                                bk_t[m_i])
                else:
                    nc.scalar.activation(kT[(m_i, sb)][:], ps[:],
                                         mybir.ActivationFunctionType.Identity,
                                         bias=bk_t[m_i])

            def vp_a(m_i):
                ps = mmps.tile([128, SB], F32, name="mm", tag="mm")
                cell[("v", m_i)] = ps
                for c in range(4):
                    nc.tensor.matmul(ps[:], xvh[0][:, c, m_i * 128:(m_i + 1) * 128],
                                     wkvv(c), start=(c == 0), stop=False)

            def vp_b(m_i):
                ps = cell.pop(("v", m_i))
                for c in range(4):
                    nc.tensor.matmul(ps[:], xvh[1][:, c, m_i * 128:(m_i + 1) * 128],
                                     wkvv(c + 4), start=False, stop=(c == 3))
                t = vt[sb * 4 + m_i]
                t3 = t[:].rearrange("p (h c) -> p h c", h=NH)
                nc.vector.tensor_tensor(
                    t3[:, :, 0:DKH],
                    ps[:].rearrange("p (h c) -> p h c", h=NH),
                    bv_b[:].rearrange("p (h c) -> p h c", h=NH),
                    AluOpType.add,
                )
                nc.vector.memset(t3[:, :, DKH:VW], 1.0)

            def qp_a(m_i):
                ps = mmps.tile([128, SB], F32, name="mm", tag="mm")
                cell[("q", m_i)] = ps
                for c in range(4):
                    nc.tensor.matmul(ps[:], wqc(c)[:, m_i * 128:(m_i + 1) * 128],
                                     xqh[0][:, c, :], start=(c == 0), stop=False)

            def qp_b(m_i):
                ps = cell.pop(("q", m_i))
                for c in range(4):
                    nc.tensor.matmul(ps[:], wqc(c + 4)[:, m_i * 128:(m_i + 1) * 128],
                                     xqh[1][:, c, :], start=False, stop=(c == 3))
                qt = qpool.tile([128, SB], BF16, name=f"qT{m_i}", tag=f"qT{m_i}")
                if sb in (0, 3):
                    nc.vector.tensor_scalar_add(qt[:], ps[:], bq_t[m_i])
                else:
                    nc.scalar.activation(qt[:], ps[:],
                                         mybir.ActivationFunctionType.Identity,
                                         bias=bq_t[m_i])
                qT[(m_i, sb)] = qt

            ku, vu, qu = [], [], []
            for m_i in range(4):
                ku += [lambda m_i=m_i: kp_a(m_i), lambda m_i=m_i: kp_b(m_i)]
                vu += [lambda m_i=m_i: vp_a(m_i), lambda m_i=m_i: vp_b(m_i)]
                qu += [lambda m_i=m_i: qp_a(m_i), lambda m_i=m_i: qp_b(m_i)]
            return ku, vu, qu

        def make_proj_units(p_sb, p_at):
            units = []

            def u(m_i, n_i):
                ps = mmps.tile([128, SB], F32, name="yps", tag="mm")
                for hp in range(NHP):
                    nc.tensor.matmul(
                        ps[:],
                        p_at[hp][:, m_i * 128:(m_i + 1) * 128],
                        woh2(hp)[:, n_i * SB:(n_i + 1) * SB],
                        start=(hp == 0),
                        stop=(hp == NHP - 1),
                    )
                yt = ysb.tile([128, SB], BF16, name="yt", tag="y")
                nc.vector.tensor_tensor(yt[:], ps[:],
                                        bo_b[:, n_i * SB:(n_i + 1) * SB],
                                        AluOpType.add)
                eng = nc.sync if n_i == 0 else nc.gpsimd
                eng.dma_start(
                    y[p_sb * SB + m_i * 128: p_sb * SB + (m_i + 1) * 128,
                      n_i * SB:(n_i + 1) * SB],
                    yt[:],
                )

            for m_i in range(4):
                for n_i in range(2):
                    units.append(lambda m_i=m_i, n_i=n_i: u(m_i, n_i))
            return units

        def make_projA_units(p_at, ya_cell):
            # first half of the last block's Wo projection (hp 0+1), folded
            # with the bo bias into f32 partials; runs as late-window filler
            units = []

            def u(m_i, n_i):
                ps = mmps.tile([128, SB], F32, name="yps", tag="mm")
                for hp in range(3):
                    nc.tensor.matmul(
                        ps[:],
                        p_at[hp][:, m_i * 128:(m_i + 1) * 128],
                        woh2(hp)[:, n_i * SB:(n_i + 1) * SB],
                        start=(hp == 0),
                        stop=(hp == 2),
                    )
                ya = ysb.tile([128, SB], F32, name="ya", tag="ya", bufs=8)
                nc.vector.tensor_tensor(ya[:], ps[:],
                                        bo_b[:, n_i * SB:(n_i + 1) * SB],
                                        AluOpType.add)
                ya_cell[(m_i, n_i)] = ya

            for m_i in range(4):
                for n_i in range(2):
                    units.append(lambda m_i=m_i, n_i=n_i: u(m_i, n_i))
            return units

        def emit_projB(p_sb, p_at, ya_cell):
            for m_i in range(4):
                for n_i in range(2):
                    ps = mmps.tile([128, SB], F32, name="yps", tag="mm")
                    nc.tensor.matmul(
                        ps[:],
                        p_at[3][:, m_i * 128:(m_i + 1) * 128],
                        woh2(3)[:, n_i * SB:(n_i + 1) * SB],
                        start=True,
                        stop=True,
                    )
                    yt = ysb.tile([128, SB], BF16, name="yt", tag="y")
                    nc.vector.tensor_tensor(yt[:], ps[:], ya_cell[(m_i, n_i)][:],
                                            AluOpType.add)
                    rows = slice(p_sb * SB + m_i * 128, p_sb * SB + (m_i + 1) * 128)
                    h = SB // 2
                    nc.sync.dma_start(
                        y[rows, n_i * SB:n_i * SB + h], yt[:, 0:h])
                    nc.gpsimd.dma_start(
                        y[rows, n_i * SB + h:(n_i + 1) * SB], yt[:, h:SB])

        # ---------- attention ----------
        def emit_scores_exp(sb, hp, kc):
            kts = kT[(hp, kc // 4)]
            koff = (kc % 4) * 128
            r_i = kc - 4 * sb
            qoff = 128 * r_i if r_i > 0 else 0
            psAB = scps.tile([128, 2 * SB], F32, name="sAB", tag="sc")
            nc.tensor.matmul(
                psAB[:, qoff:SB],
                kts[0:64, koff:koff + 128],
                qT[(hp, sb)][0:64, qoff:SB],
                start=True, stop=True,
            )
            nc.tensor.matmul(
                psAB[:, SB + qoff:2 * SB],
                kts[64:128, koff:koff + 128],
                qT[(hp, sb)][64:128, qoff:SB],
                start=True, stop=True,
            )
            pAB = probs.tile([128, 2 * SB], BF16, name="pAB", tag="probs")
            ps3 = psAB[:].rearrange("p (h q) -> p h q", h=2)[:, :, qoff:SB]
            pr3 = pAB[:].rearrange("p (h q) -> p h q", h=2)[:, :, qoff:SB]
            nc.scalar.activation(pr3, ps3, mybir.ActivationFunctionType.Exp,
                                 scale=0.125)
            if r_i >= 0:
                # zero the strictly-lower triangle of the 128-wide diag strip
                tr3 = pAB[:].rearrange("p (h q) -> p h q", h=2)[:, :, qoff:qoff + 128]
                mk3 = tri[:].rearrange("p (h q) -> p h q", h=2)
                nc.vector.tensor_tensor(tr3, tr3, mk3, AluOpType.mult)
            return pAB, qoff

        def emit_pv(kc, pAB, qoff, pvA, pvB, hp, nck):
            vtile = vt[kc]
            hA, hB = 2 * hp, 2 * hp + 1
            nc.tensor.matmul(
                pvA[0:VW, qoff:SB], vtile[:, hA * VW:(hA + 1) * VW],
                pAB[:, qoff:SB],
                start=(kc == 0), stop=(kc == nck - 1),
            )
            nc.tensor.matmul(
                pvB[0:VW, qoff:SB], vtile[:, hB * VW:(hB + 1) * VW],
                pAB[:, SB + qoff:2 * SB],
                start=(kc == 0), stop=(kc == nck - 1),
            )

        def emit_normalize(hp, pvA, pvB, at_tiles):
            # copy out of PSUM promptly to free the accumulators
            pvcA = nrm.tile([VW, SB], F32, name="pvcA", tag="pvc")
            nc.vector.tensor_copy(pvcA[:], pvA[0:VW, :])
            pvcB = nrm.tile([VW, SB], F32, name="pvcB", tag="pvc")
            nc.vector.tensor_copy(pvcB[:], pvB[0:VW, :])
            at = atp.tile([128, SB], BF16, name=f"at{hp}", tag=f"at{hp}")
            for h_sub, pvc in ((0, pvcA), (1, pvcB)):
                rc0 = nrm.tile([1, SB], F32, name="rc0", tag="rc0", bufs=2)
                nc.sync.dma_start(rc0[:], pvc[64:65, :])
                rc1 = nrm.tile([1, SB], F32, name="rc1", tag="rc1", bufs=2)
                nc.vector.reciprocal_approx_fast(rc1[:], rc0[:])
                rb = nrm.tile([64, SB], F32, name="rb", tag="rb", bufs=2)
                nc.gpsimd.partition_broadcast(rb[:], rc1[:])
                if h_sub == 0:
                    nc.vector.tensor_tensor(at[0:64, :], pvc[0:64, :], rb[:],
                                            AluOpType.mult)
                else:
                    ato = atp.tile([64, SB], BF16, name="ato", tag="ato")
                    nc.vector.tensor_tensor(ato[:], pvc[0:64, :], rb[:],
                                            AluOpType.mult)
                    nc.gpsimd.dma_start(at[64:128, :], ato[:])
            at_tiles[hp] = at

        def emit_normalize_last(hp, pvA, pvB, at_tiles):
            # tail fast path: denominators straight from PSUM, broadcast on
            # the (idle) tensor engine, multiplies read PSUM directly
            at = atp.tile([128, SB], BF16, name=f"at{hp}", tag=f"at{hp}")
            # copy the PSUM denominator rows (lane 64) to SBUF, broadcast
            # raw on the (idle) tensor engine via a K=1 matmul, take the
            # reciprocal of the broadcast (PSUM lanes 0:64 - the custom DVE
            # recip needs a partition-0-aligned input), then normalize
            # reading the PV accumulators straight out of PSUM.
            rowt = nrm.tile([65, 2 * SB], BF16, name="rowt", tag="rowt", bufs=1)
            nc.scalar.copy(rowt[64:65, 0:SB], pvA[64:65, :])
            nc.scalar.copy(rowt[64:65, SB:2 * SB], pvB[64:65, :])
            bc = scps.tile([128, 2 * SB], F32, name="bc", tag="sc")
            nc.tensor.matmul(bc[0:64, 0:SB], ones_bc[64:65, :],
                             rowt[64:65, 0:SB], start=True, stop=True)
            nc.tensor.matmul(bc[0:64, SB:2 * SB], ones_bc[64:65, :],
                             rowt[64:65, SB:2 * SB], start=True, stop=True)
            rr = nrm.tile([64, 2 * SB], F32, name="rr", tag="rr", bufs=1)
            nc.vector.reciprocal_approx_fast(rr[:, 0:SB], bc[0:64, 0:SB])
            nc.vector.reciprocal_approx_fast(rr[:, SB:2 * SB],
                                             bc[0:64, SB:2 * SB])
            nc.vector.tensor_tensor(at[0:64, :], pvA[0:64, :], rr[:, 0:SB],
                                    AluOpType.mult)
            ato = atp.tile([64, SB], BF16, name="ato", tag="ato")
            nc.vector.tensor_tensor(ato[:], pvB[0:64, :], rr[:, SB:2 * SB],
                                    AluOpType.mult)
            nc.sync.dma_start(at[64:128, :], ato[:])
            at_tiles[hp] = at

        # ---------- main schedule ----------
        # prologue: x(0) loads + QKV(0) back to back; the deferred constants
        # ride the sync queue right behind the x tiles
        xk0 = load_x(xk, "xk", 0)
        nc.sync.dma_start(tri[:], tri_d[:])
        nc.sync.dma_start(bv_b[:], bvb_d[:])
        xv0 = load_x(xv, "xv", 0)
        xq0 = load_x(xq, "xq", 0)
        nc.sync.dma_start(bo_b[:], bob_d[:])
        ku, vu, qu = make_qkv_units(0, xk0, xv0, xq0)
        for u in ku + vu + qu:
            u()

        # window filler assignment (sb -> list of unit lists):
        #   W0: QKV(1);  W1: QKV(2) + PROJ(0);  W2: q(3) + PROJ(1);
        #   W3: k(3) + v(3) front-loaded + PROJ(2); tail: PROJ(3)
        kv3_units = None
        prev_at = None
        for sb in range(NSB):
            if sb == 0:
                # startup burst is over: fetch the deferred wo
                nc.scalar.dma_start(
                    wot_t[:].rearrange("p (c d) -> p c d", c=4),
                    wo3[:, :, :].rearrange("c p d -> p c d"))
            front = []
            fillers = []
            if sb + 1 < NSB:
                xkt = load_x(xk, "xk", sb + 1)
                xvt = load_x(xv, "xv", sb + 1)
                xqt = load_x(xq, "xq", sb + 1)
                ku, vu, qu = make_qkv_units(sb + 1, xkt, xvt, xqt)
                if sb + 1 < NSB - 1:
                    fillers += ku + vu + qu
                else:
                    # defer k(3)/v(3) into W3 to keep its tensor queue fed
                    fillers += qu
                    kv3_units = [u for pair in zip(ku, vu) for u in pair]
            else:
                front = kv3_units
            if prev_at is not None:
                fillers += make_proj_units(sb - 1, prev_at)

            nck = 4 * sb + 4
            n_chunks = NHP * nck
            chunk_j = 0
            popped = 0
            total_units = len(fillers)
            ya_cell = {}

            at_tiles = [None] * NHP
            for hp in range(NHP):
                pvA = pvps.tile([128, SB], F32, name="pvA", tag="pv")
                pvB = pvps.tile([128, SB], F32, name="pvB", tag="pv")
                pending = emit_scores_exp(sb, hp, 0)
                for kc in range(nck):
                    nxt = emit_scores_exp(sb, hp, kc + 1) if kc + 1 < nck else None
                    chunk_j += 1
                    # front-loaded units: 2 per chunk until drained
                    if front:
                        front.pop(0)()
                        if front:
                            front.pop(0)()
                    # spread fillers evenly over the window's chunks; in the
                    # last window hold a few back so they drain during the
                    # tail-normalize chain instead of leaving tensor idle
                    eff_j = chunk_j if sb < NSB - 1 else max(0, chunk_j - 12)
                    while fillers and popped * n_chunks < eff_j * total_units:
                        fillers.pop(0)()
                        popped += 1
                    pAB, qoff = pending
                    emit_pv(kc, pAB, qoff, pvA, pvB, hp, nck)
                    pending = nxt
                if sb == NSB - 1 and hp == NHP - 1:
                    emit_normalize_last(hp, pvA, pvB, at_tiles)
                else:
                    emit_normalize(hp, pvA, pvB, at_tiles)
                if sb == NSB - 1 and hp == 2:
                    new_units = make_projA_units(at_tiles, ya_cell)
                    fillers += new_units
                    total_units += len(new_units)
            for u in front + fillers:
                u()
            prev_at = at_tiles

        emit_projB(NSB - 1, prev_at, ya_cell)

    nc.compile()
    return nc


_NC = None
_LAST_IN_MAPS = None


def _get_nc():
    global _NC
    if _NC is None:
        _NC = build_program()
    return _NC


def kernel(query, key, value, mask, Wq, bq, Wk, bk, Wv, bv, Wo, bo):
    query = np.asarray(query, np.float32)
    key = np.asarray(key, np.float32)
    value = np.asarray(value, np.float32)
    Wq = np.asarray(Wq, np.float32)
    Wk = np.asarray(Wk, np.float32)
    Wv = np.asarray(Wv, np.float32)
    Wo = np.asarray(Wo, np.float32)
    bq = np.asarray(bq, np.float32)
    bk = np.asarray(bk, np.float32)
    bv = np.asarray(bv, np.float32)
    bo = np.asarray(bo, np.float32)

    nc = _get_nc()

    B = query.shape[0]
    bf = ml_dtypes.bfloat16
    triu = np.triu(np.ones((128, 128), np.float32))
    tri_b = np.ascontiguousarray(
        np.concatenate([triu, triu], axis=1).astype(bf))
    xq_t = [np.ascontiguousarray(query[b].T.astype(bf)) for b in range(B)]
    xk_t = [np.ascontiguousarray(key[b].T.astype(bf)) for b in range(B)]
    xv_t = [np.ascontiguousarray(value[b].T.astype(bf)) for b in range(B)]

    in_maps = []
    for c in range(8):
        b, hg = c // 2, c % 2
        sl = slice(hg * 512, (hg + 1) * 512)
        in_maps.append({
            "xq_t": xq_t[b],
            "xk_t": xk_t[b],
            "xv_t": xv_t[b],
            "wq_t": np.ascontiguousarray(Wq[sl, :].T.astype(bf)),
            "wkv_t": np.ascontiguousarray(
                np.concatenate([Wk[sl, :].T, Wv[sl, :].T], axis=1).astype(bf)),
            "wo_t": np.ascontiguousarray(Wo[:, sl].T.astype(bf)),
            "bq_s": np.ascontiguousarray(bq[sl].reshape(4, 128).T),
            "bk_s": np.ascontiguousarray(bk[sl].reshape(4, 128).T),
            "bv_b": np.ascontiguousarray(
                np.broadcast_to(bv[None, sl], (128, 512)).astype(bf)),
            "bo_b": np.ascontiguousarray(
                np.broadcast_to(bo[None, :] * 0.5, (128, D)).astype(bf)),
            "tri_b": tri_b,
        })

    global _LAST_IN_MAPS
    _LAST_IN_MAPS = in_maps
    res = run_bass_kernel_spmd(nc, in_maps, core_ids=list(range(8)))
    out = np.empty((B, S, D), np.float32)
    for b in range(B):
        out[b] = (res.results[2 * b]["y"].astype(np.float32)
                  + res.results[2 * b + 1]["y"].astype(np.float32))
    return out
